# revision 1
# baseline (speedup 1.0000x reference)
"""ConvKAN Trainium2 Bass kernel.

Problem: nn_ConvKAN (B=8, C=64, H=W=64, OUT=64, 3x3 conv, KAN spline G=5 k=3).

Algorithm (per core, data-parallel over batch: core i handles image i):
  The ConvKAN is a 3x3 convolution over a channel-expanded input:
    out[o,y,x] = sum_{c,kh,kw} [ sum_j w_spline[o,(c,kh,kw),j] * B_j(xp[c,y+kh,x+kw])
                                 + w_base[o,(c,kh,kw)] * silu(xp[c,y+kh,x+kw]) ]
  with xp the zero-padded input. On the uniform knot grid the cubic B-spline
  basis has the bounded two-sided form (exact, no cancellation blowup):
    tau = (x + 2.2)/0.4 ;  u_j = |tau - j - 2|
    B_j = relu(2 - u_j)^3 / 6 - (4/6) * relu(1 - u_j)^3
  Each core computes 8 basis channels + 1 silu channel per input channel in
  SBUF (fp16), then performs the conv as 9 shift-offset matmuls (K = 9*64
  channels in 5 K-tiles) accumulated in PSUM, using PE column-group packing
  (two concurrent M=64 matmuls on col-groups 0-1 / 2-3) to fill the array.
"""
import os
import sys

sys.path.insert(0, "/opt/trn_rl_repo")

import numpy as np

import concourse.bass as bass
import concourse.bacc as bacc
import concourse.tile as tile
from concourse import mybir
from concourse.bass_utils import run_bass_kernel_spmd

# ---- problem constants (hardcoded per contest rules) ----
B, C, H, W = 8, 64, 64, 64
OUT_CH = 64
NJ = 8                      # spline basis functions per feature
HP, WP = H + 2, W + 2       # padded spatial
S = HP * WP                 # 4356
RW = S + 2                  # R tile width: lead+tail pad cell, data at +1
N_STRIPS = 6
STRIP = S // N_STRIPS       # 726 = 11 padded rows
GRID_H = 0.4
TAU_SCALE = 1.0 / GRID_H    # 2.5
# u_j = |x - cj|, cj = (j - 3.5)/2.5  (x-units);  a = relu(2 - 2.5*u)
F32 = mybir.dt.float32
F16 = mybir.dt.float16

# chunks of output rows for the matmul stage (N = rows*66 <= 462 fits PSUM bank)
CHUNKS = [(y0, 7) for y0 in range(0, 63, 7)] + [(63, 1)]  # 10 chunks
assert sum(r for _, r in CHUNKS) == H

_CACHE = {}


def _fold_weights(base_weight, spline_weight, spline_scaler):
    """Host-side weight prep into lhsT layout [128, 45*64] fp16.

    Channel layout (contraction dim): k-tile t in 0..3 holds basis channels
    j=2t (partitions 0:64, c-major) and j=2t+1 (partitions 64:128); k-tile 4
    holds the silu channel (partitions 0:64). Block index = (kh*3+kw)*5 + t.
    """
    sw = (spline_weight.astype(np.float64) * spline_scaler.astype(np.float64)[:, :, None])
    sw4 = sw.reshape(OUT_CH, C, 9, NJ)           # o, c, s9, j
    bw4 = base_weight.astype(np.float64).reshape(OUT_CH, C, 9)  # o, c, s9
    Wk = np.zeros((128, 45, 64), np.float64)
    for s9 in range(9):
        for t in range(4):
            for half in range(2):
                j = 2 * t + half
                Wk[half * 64:(half + 1) * 64, s9 * 5 + t, :] = sw4[:, :, s9, j].T
        Wk[0:64, s9 * 5 + 4, :] = bw4[:, :, s9].T
    return Wk.reshape(128, 45 * 64).astype(np.float16)


def _ctab():
    ct = np.zeros((128, 10), np.float32)
    for t in range(4):
        for p in range(128):
            j = 2 * t + p // 64
            ct[p, t] = (j - 3.5) / 2.5
    ct[:, 4] = 2.0   # relu bias column (a)
    ct[:, 5] = -1.0  # relu bias column (b = relu(a-1))
    ct[:, 6:10] = -ct[:, 0:4]  # negated centers: u = Abs(x + (-cj))
    return ct


def _build_nc():
    nc = bacc.Bacc()
    x_ext = nc.dram_tensor("x_img", [C, H, W], F32, kind="ExternalInput")
    wk_ext = nc.dram_tensor("wk", [128, 45 * 64], F16, kind="ExternalInput")
    ct_ext = nc.dram_tensor("ct", [128, 10], F32, kind="ExternalInput")
    out_ext = nc.dram_tensor("out", [OUT_CH, H, W], F32, kind="ExternalOutput")

    with tile.TileContext(nc) as tc:
        with (
            tc.tile_pool(name="const", bufs=1) as const_pool,
            tc.tile_pool(name="temps", bufs=2) as temp_pool,
            tc.tile_pool(name="outs", bufs=4) as out_pool,
            tc.tile_pool(name="psum", bufs=4, space="PSUM") as psum_pool,
        ):
            # ---- constants / inputs to SBUF ----
            wsb = const_pool.tile([128, 45 * 64], F16, tag="wsb")
            nc.sync.dma_start(out=wsb[:, :], in_=wk_ext[:, :])
            ctab = const_pool.tile([128, 10], F32, tag="ctab")
            nc.sync.dma_start(out=ctab[:, :], in_=ct_ext[:, :])

            xx = const_pool.tile([128, S], F32, tag="xx")
            nc.vector.memset(xx[:, :], 0.0)
            xx3 = xx[:, :].rearrange("p (h w) -> p h w", w=WP)
            nc.sync.dma_start(out=xx3[0:64, 1:65, 1:65], in_=x_ext[:, :, :])
            nc.sync.dma_start(out=xx3[64:128, 1:65, 1:65], in_=x_ext[:, :, :])

            # ---- basis channel tiles (fp16) ----
            rts = [const_pool.tile([128, RW], F16, tag=f"r{t}", name=f"r{t}") for t in range(4)]
            rsilu = const_pool.tile([64, RW], F16, tag="rsilu")
            # NOTE: R cells 0 and RW-1 are never written (only read for the
            # discarded xp=0/65 garbage PSUM columns) -- garbage is fine there.

            # ---- elementwise stage: basis + silu channels, strip-mined ----
            for sp in range(N_STRIPS):
                s0, s1 = sp * STRIP, (sp + 1) * STRIP
                xs_full = xx[:, s0:s1]
                for t in range(4):
                    # u = |x - cj|  (ACT Abs with per-partition bias -cj)
                    u = temp_pool.tile([128, STRIP], F16, tag="u")
                    nc.scalar.activation(u[:, :], xs_full,
                                         mybir.ActivationFunctionType.Abs,
                                         bias=ctab[:, 6 + t:7 + t], scale=1.0)
                    # a = relu(2 - 2.5u)
                    a = temp_pool.tile([128, STRIP], F16, tag="a")
                    nc.scalar.activation(a[:, :], u[:, :],
                                         mybir.ActivationFunctionType.Relu,
                                         bias=ctab[:, 4:5], scale=-TAU_SCALE)
                    # b = relu(a - 1) = max(a, 1) - 1   (DVE, fp16 2x mode)
                    b = temp_pool.tile([128, STRIP], F16, tag="b")
                    nc.vector.tensor_scalar(
                        b[:, :], a[:, :], 1.0, -1.0,
                        mybir.AluOpType.max, mybir.AluOpType.add)
                    # a2 = a*a, b2 = b*b (DVE fp16)
                    a2 = temp_pool.tile([128, STRIP], F16, tag="a2")
                    nc.vector.tensor_mul(a2[:, :], a[:, :], a[:, :])
                    b2 = temp_pool.tile([128, STRIP], F16, tag="b2")
                    nc.vector.tensor_mul(b2[:, :], b[:, :], b[:, :])
                    # s1 = (a2/6)*a ; s2 = (-2/3 b2)*b ; Bj = s1+s2 -> fp16
                    s1t = temp_pool.tile([128, STRIP], F16, tag="s1")
                    nc.vector.scalar_tensor_tensor(
                        s1t[:, :], a2[:, :], 1.0 / 6.0, a[:, :],
                        mybir.AluOpType.mult, mybir.AluOpType.mult)
                    s2t = temp_pool.tile([128, STRIP], F16, tag="s2")
                    nc.vector.scalar_tensor_tensor(
                        s2t[:, :], b2[:, :], -2.0 / 3.0, b[:, :],
                        mybir.AluOpType.mult, mybir.AluOpType.mult)
                    nc.vector.tensor_add(rts[t][:, 1 + s0:1 + s1], s1t[:, :], s2t[:, :])
                # silu channel
                nc.scalar.activation(rsilu[:, 1 + s0:1 + s1], xx[0:64, s0:s1],
                                     mybir.ActivationFunctionType.Silu)

            # ---- conv: 9 shifts x 5 K-tiles, col-group-packed matmul pairs ----
            for cp in range(5):
                (y0e, re_), (y0o, ro_) = CHUNKS[2 * cp], CHUNKS[2 * cp + 1]
                ne, no = re_ * WP, ro_ * WP
                ps = psum_pool.tile([128, 462], F32, tag="ps")
                for s9 in range(9):
                    kh, kw = s9 // 3, s9 % 3
                    for t in range(5):
                        kdim = 128 if t < 4 else 64
                        rt = rts[t] if t < 4 else rsilu
                        blk = (s9 * 5 + t) * 64
                        first = (s9 == 0 and t == 0)
                        last = (s9 == 8 and t == 4)
                        offe = (y0e + kh) * WP + kw
                        nc.tensor.matmul(
                            ps[0:64, 0:ne],
                            wsb[0:kdim, blk:blk + 64],
                            rt[0:kdim, offe:offe + ne],
                            start=first, stop=last, tile_position=(0, 0))
                        offo = (y0o + kh) * WP + kw
                        nc.tensor.matmul(
                            ps[64:128, 0:no],
                            wsb[0:kdim, blk:blk + 64],
                            rt[0:kdim, offo:offo + no],
                            start=first, stop=last, tile_position=(0, 64))
                # drain psum -> sbuf (ScalarE is close to PSUM), then DMA out
                oe = out_pool.tile([64, 462], F32, tag="oe")
                nc.scalar.copy(oe[:, 0:ne], ps[0:64, 0:ne])
                oev = oe[:, 0:ne].rearrange("p (r w) -> p r w", w=WP)
                nc.sync.dma_start(out=out_ext[:, y0e:y0e + re_, :],
                                  in_=oev[:, :, 1:65])
                oo = out_pool.tile([64, 462], F32, tag="oo")
                nc.scalar.copy(oo[:, 0:no], ps[64:128, 0:no])
                oov = oo[:, 0:no].rearrange("p (r w) -> p r w", w=WP)
                nc.sync.dma_start(out=out_ext[:, y0o:y0o + ro_, :],
                                  in_=oov[:, :, 1:65])
    nc.finalize()
    return nc


def kernel(x, base_weight, spline_weight, spline_scaler):
    x = np.ascontiguousarray(np.asarray(x, dtype=np.float32))
    wk = _fold_weights(np.asarray(base_weight), np.asarray(spline_weight),
                       np.asarray(spline_scaler))
    ct = _ctab()

    if "nc" not in _CACHE:
        _CACHE["nc"] = _build_nc()
    nc = _CACHE["nc"]

    in_maps = [{"x_img": x[i], "wk": wk, "ct": ct} for i in range(B)]
    res = run_bass_kernel_spmd(nc, in_maps, list(range(B)))
    _CACHE["last_res"] = res
    out = np.stack([res.results[i]["out"] for i in range(B)], axis=0)
    return out.astype(np.float32)


if __name__ == "__main__":
    rng = np.random.default_rng(0)
    ins = {
        "x": rng.standard_normal((B, C, H, W), dtype=np.float32),
        "base_weight": (rng.standard_normal((OUT_CH, 576)) * 0.05).astype(np.float32),
        "spline_weight": (rng.standard_normal((OUT_CH, 576, NJ)) * 0.05).astype(np.float32),
        "spline_scaler": (rng.standard_normal((OUT_CH, 576)) * 0.05).astype(np.float32),
    }
    o = kernel(**ins)
    print("kernel out:", o.shape, o.dtype, float(np.abs(o).max()))



# revision 10
# speedup vs baseline: 1.4082x; 1.4082x over previous
"""ConvKAN Trainium2 Bass kernel (v2).

Problem: nn_ConvKAN (B=8, C=64, H=W=64, OUT=64, 3x3 conv, KAN spline G=5 k=3).

Data-parallel over batch: core i handles image i. Per core:
  out[o,y,x] = sum_{c,kh,kw} [ sum_j w6[o,(c,kh,kw),j] * (6*B_j)(xp[c,y+kh,x+kw])
                               + w_base[o,(c,kh,kw)] * silu(xp[c,y+kh,x+kw]) ]
  with xp the zero-padded input and the exact two-sided cubic bump
    c_j = (j-3.5)*0.4 ; u = |x-c_j| ; a = relu(2-2.5u) ; r = relu(a-1)
    6*B_j(x) = a^3 - 4 r^3            (weights pre-divided by 6 host-side)

Design notes:
  - x loads packed/contiguous ([64,4096] rows, 16KB per partition) twice
    (both partition halves), basis computed from packed x, written into the
    padded (66-col rows) basis layout; pad cells pre-filled with the constant
    6*B_j(0) per partition (and 0 for the silu plane).
  - Elementwise split across three engines: ACT (u for t<2, a, silu),
    DVE (u for t>=2, r, a^3 and 4r^3 via tensor_scalar pow), GPSIMD
    (fp32->fp16 cast, silu shift plane, final subtract for t>=2, drains).
  - Conv as shift-offset matmuls accumulated in PSUM, two col-groups
    (tile_position (0,0)/(0,64)) for concurrent M=64 chunks, and fp16
    DoubleRow perf mode packing two K=128 contraction slabs per matmul.
"""
import os
import sys

sys.path.insert(0, "/opt/trn_rl_repo")

import numpy as np

import concourse.bass as bass
import concourse.bacc as bacc
import concourse.tile as tile
from concourse import mybir
from concourse.ap import AP
from concourse.bass_utils import run_bass_kernel_spmd

# ---- problem constants (hardcoded per contest rules) ----
B, C, H, W = 8, 64, 64, 64
OUT_CH = 64
NJ = 8
WP = 66                    # padded row width
S = WP * WP                # 4356 padded cells
RW = S + 2                 # basis tile width (matmul tail reads 2 cells past)
NSTRIP = 4
SR = 16                    # image rows per strip
PK = SR * W                # packed cols per strip (1024)

F32 = mybir.dt.float32
F16 = mybir.dt.float16
ALU = mybir.AluOpType
ACTF = mybir.ActivationFunctionType

# chunks of output rows (N = rows*66 <= 462 fits a PSUM bank)
CHUNKS = [(7 * i, 7) for i in range(9)] + [(63, 1)]

USE_DR = False      # DoubleRow matmul packing (fp8-only on TRN2 -> off)
ACT_PLANES = (0, 1)  # planes via ACT Abs+Relu; rest via custom DVE A_FROM_X

_CACHE = {}


def _register_dve_ops():
    """Register the two fused ConvKAN ops in dve_ops.OPS (idempotent)."""
    if "ops" in _CACHE:
        return _CACHE["ops"]
    import concourse.dve_ops as DO
    from concourse.dve_spec import (
        Spec, Src0, C0, C1, C2, One, relu, sq, minn, lower)
    from concourse.dve_uop import DveOpSpec

    def make(name, spec):
        for op in DO.OPS:
            if op.name == name:
                return op
        row = max(DO._SUB_OPCODE_FOR_NAME.values()) + 1
        shas = {}
        for ver in ("v3", "v4"):
            try:
                s = DveOpSpec(name=name, opcode=row, uops=lower(spec, ver=ver),
                              rd1_en=False)
                shas[ver] = s.sha(ver)
            except Exception:
                pass
        op = DO.DveOp(name, spec, subdim=False, uops_sha=shas)
        DO.OPS.append(op)
        DO.CUSTOM_DVE_SPECS[name] = spec
        DO._SUB_OPCODE_FOR_NAME[name] = row
        return op

    d = Src0 - C0
    m1 = d * C1
    a_spec = Spec(
        body=minn(relu(m1 + C2), relu(C2 - m1)),
        reference=lambda in0, in1, s0, s1, imm2: np.minimum(
            np.maximum((in0 - s0) * s1 + imm2, 0),
            np.maximum(imm2 - (in0 - s0) * s1, 0)))
    r = relu(Src0 - One)
    b6_spec = Spec(
        body=sq(Src0) * Src0 + C0 * (sq(r) * r),
        reference=lambda in0, in1, s0, s1, imm2: (
            in0.astype(np.float32) ** 3
            + s0 * np.maximum(in0.astype(np.float32) - 1.0, 0) ** 3))
    ops = (make("CONVKAN_A_ANT", a_spec), make("CONVKAN_B6_ANT", b6_spec))
    _CACHE["ops"] = ops
    return ops


def _centers():
    # per-partition center for plane t: j = 2t + (p//64)
    c = np.zeros((128, 4), np.float64)
    for t in range(4):
        for p in range(128):
            j = 2 * t + p // 64
            c[p, t] = (j - 3.5) * 0.4
    return c


def _mm_entries():
    """Static matmul plan per chunk: list of dicts.

    kind 'dr': DoubleRow, two K=128 slabs; kind 's': regular K=128 single.
    tile: 0..3 (basis plane) or 4 (silu). offs: slab offsets relative to
    y0*66 (ascending for dr). Weight layout cursor advances 64 cols/slab.
    """
    ent = []
    off = lambda kh, kw: kh * WP + kw
    for t in range(4):
        pairs = [((0, 0), (1, 0)), ((0, 1), (2, 0)), ((1, 1), (2, 1)),
                 ((0, 2), (1, 2))]
        for (ka, kb) in pairs:
            o1, o2 = off(*ka), off(*kb)
            assert o1 < o2
            ent.append(dict(kind="dr", tile=t, offs=(o1, o2), taps=(ka, kb)))
        ent.append(dict(kind="s", tile=t, offs=(off(2, 2),), taps=((2, 2),)))
    # silu slabs on the shifted-duplicate tile: slab@o covers taps o (top
    # half, c-major) and o+1 (bottom half). kw=2 slabs have zero bottom.
    ent.append(dict(kind="dr", tile=4, offs=(off(0, 0), off(1, 0)),
                    taps=((0, 0), (1, 0))))
    ent.append(dict(kind="dr", tile=4, offs=(off(0, 2), off(2, 0)),
                    taps=((0, 2), (2, 0))))
    ent.append(dict(kind="dr", tile=4, offs=(off(1, 2), off(2, 2)),
                    taps=((1, 2), (2, 2))))
    return ent


ENTRIES = _mm_entries()
NWCOL = sum(128 if e["kind"] == "dr" else 64 for e in ENTRIES)


def _fold_weights(base_weight, spline_weight, spline_scaler):
    """Host-side weight prep into lhsT layout [128, NWCOL] fp16."""
    sw6 = (spline_weight.astype(np.float64)
           * spline_scaler.astype(np.float64)[:, :, None]) / 6.0
    sw6 = sw6.reshape(OUT_CH, C, 9, NJ)          # o, c, s9, j
    bw = base_weight.astype(np.float64).reshape(OUT_CH, C, 9)
    wk = np.zeros((128, NWCOL), np.float64)
    cur = 0
    for e in ENTRIES:
        for (kh, kw) in e["taps"]:
            s9 = kh * 3 + kw
            blk = wk[:, cur:cur + 64]
            if e["tile"] < 4:
                t = e["tile"]
                blk[0:64, :] = sw6[:, :, s9, 2 * t].T       # j=2t, c-major
                blk[64:128, :] = sw6[:, :, s9, 2 * t + 1].T
            else:
                blk[0:64, :] = bw[:, :, s9].T               # tap (kh,kw)
                if kw < 2:   # bottom half covers tap (kh,kw+1)
                    blk[64:128, :] = bw[:, :, kh * 3 + kw + 1].T
                # kw==2: bottom stays zero (shifted tap kw+1=3 is garbage)
            cur += 64
    assert cur == NWCOL
    return wk.astype(np.float16)


def _ctab():
    """[128, 13] f32: cols 0-3 centers c_j, 4-7 negated centers, 8-11 border
    constants 6*B_j(0), 12 the Relu bias 2.0."""
    ct = np.zeros((128, 13), np.float32)
    c = _centers()
    ct[:, 0:4] = c
    ct[:, 4:8] = -c
    u0 = np.abs(c)
    a0 = np.maximum(0.0, 2.0 - 2.5 * u0)
    r0 = np.maximum(0.0, a0 - 1.0)
    ct[:, 8:12] = (a0 ** 3 - 4.0 * r0 ** 3).astype(np.float32)
    ct[:, 12] = 2.0
    return ct


def _build_nc():
    nc = bacc.Bacc()
    x_ext = nc.dram_tensor("x_img", [C, H, W], F32, kind="ExternalInput")
    wk_ext = nc.dram_tensor("wk", [128, NWCOL], F16, kind="ExternalInput")
    ct_ext = nc.dram_tensor("ct", [128, 13], F32, kind="ExternalInput")
    out_ext = nc.dram_tensor("out", [OUT_CH, H, W], F32, kind="ExternalOutput")

    with tile.TileContext(nc) as tc:
        with (
            tc.tile_pool(name="const", bufs=1) as cpool,
            tc.tile_pool(name="temps", bufs=2) as tpool,
            tc.tile_pool(name="outs", bufs=4) as opool,
            tc.tile_pool(name="psum", bufs=4, space="PSUM") as ppool,
        ):
            wsb = cpool.tile([128, NWCOL], F16, tag="wsb")
            nc.sync.dma_start(out=wsb[:, :], in_=wk_ext[:, :])
            ctab = cpool.tile([128, 13], F32, tag="ctab")
            nc.sync.dma_start(out=ctab[:, :], in_=ct_ext[:, :])

            xx = cpool.tile([128, H * W], F32, tag="xx")      # packed input
            rts = [cpool.tile([128, RW], F16, tag=f"r{t}", name=f"r{t}")
                   for t in range(4)]
            ssil = cpool.tile([128, RW], F16, tag="ssil")
            z = cpool.tile([128, 132], F32, tag="z")

            xv = x_ext[:, :, :].rearrange("c h w -> c (h w)")
            for s in range(NSTRIP):
                sl = slice(s * PK, (s + 1) * PK)
                nc.sync.dma_start(out=xx[0:64, sl], in_=xv[:, sl])
                nc.sync.dma_start(out=xx[64:128, sl], in_=xv[:, sl])

            # ---- border pre-fill ----
            nc.gpsimd.memset(z[:, :], 0.0)
            for t in range(4):
                rt = rts[t]
                b0 = ctab[:, 8 + t:9 + t]
                # rows 0 and 65 (full 66 cols each)
                rows = AP(rt[:, 0:1].tensor, 1,
                          [[RW, 128], [65 * WP, 2], [1, WP]])
                nc.scalar.activation(rows, z[:, :].rearrange(
                    "p (a b) -> p a b", b=WP), ACTF.Identity,
                    bias=b0, scale=0.0)
                # cols 0 and 65 of rows 1..64
                cols = AP(rt[:, 0:1].tensor, 1 + WP,
                          [[RW, 128], [WP, 64], [65, 2]])
                nc.scalar.activation(cols, z[:, 0:128].rearrange(
                    "p (a b) -> p a b", b=2), ACTF.Identity,
                    bias=b0, scale=0.0)
                nc.gpsimd.memset(rt[:, 0:1], 0.0)
                nc.gpsimd.memset(rt[:, S + 1:RW], 0.0)
            # silu plane borders are zero
            srows = AP(ssil[:, 0:1].tensor, 1, [[RW, 128], [65 * WP, 2], [1, WP]])
            nc.gpsimd.memset(srows, 0.0)
            scols = AP(ssil[:, 0:1].tensor, 1 + WP, [[RW, 128], [WP, 64], [65, 2]])
            nc.gpsimd.memset(scols, 0.0)
            nc.gpsimd.memset(ssil[:, 0:1], 0.0)
            nc.gpsimd.memset(ssil[:, S + 1:RW], 0.0)

            rtS = [rt[:, 1:S + 1].rearrange("p (h w) -> p h w", w=WP)
                   for rt in rts]
            ssT = ssil[:, 1:S + 1].rearrange("p (h w) -> p h w", w=WP)

            A_OP, B6_OP = _register_dve_ops()

            def emit_b6(t, a, r0, r1):
                nc.vector._custom_dve(
                    B6_OP, out=rtS[t][:, r0:r1, 1:65],
                    in0=a[:, :].rearrange("p (r w) -> p r w", w=W),
                    s0=-4.0)

            def emit_strip(s):
                sl = slice(s * PK, (s + 1) * PK)
                r0, r1 = 1 + SR * s, 1 + SR * (s + 1)
                # custom-DVE planes first so the DVE queue never stalls on ACT
                for t in (2, 3):
                    a = tpool.tile([128, PK], F16, tag=f"ac{t}")
                    nc.vector._custom_dve(A_OP, out=a[:, :], in0=xx[:, sl],
                                          s0=ctab[:, t:t + 1], s1=-2.5,
                                          imm2=2.0)
                    emit_b6(t, a, r0, r1)
                for t in ACT_PLANES:
                    u = tpool.tile([128, PK], F16, tag="u")
                    nc.scalar.activation(u[:, :], xx[:, sl], ACTF.Abs,
                                         bias=ctab[:, 4 + t:5 + t],
                                         scale=1.0)
                    a = tpool.tile([128, PK], F16, tag=f"aa{t}")
                    nc.scalar.activation(a[:, :], u[:, :], ACTF.Relu,
                                         bias=ctab[:, 12:13], scale=-2.5)
                    emit_b6(t, a, r0, r1)
                # silu top half (interior)
                nc.scalar.activation(
                    ssT[0:64, r0:r1, 1:65],
                    xx[0:64, sl].rearrange("p (r w) -> p r w", w=W),
                    ACTF.Silu)
                # silu bottom half: flat shift-by-one copy of the top
                c0 = 1 + r0 * WP
                nc.gpsimd.tensor_copy(
                    ssil[64:128, c0:c0 + SR * WP],
                    ssil[0:64, c0 + 1:c0 + 1 + SR * WP])

            def emit_pair(p):
                ps = ppool.tile([128, 462], F32, tag="ps")
                outs = []
                for ci in range(2):
                    y0, nr = CHUNKS[2 * p + ci]
                    ne = nr * WP
                    pos = (0, 0) if ci == 0 else (0, 64)
                    psl = ps[0:64, 0:ne] if ci == 0 else ps[64:128, 0:ne]
                    cur = 0
                    for ei, e in enumerate(ENTRIES):
                        rt = rts[e["tile"]] if e["tile"] < 4 else ssil
                        o1 = y0 * WP + e["offs"][0]
                        if e["kind"] == "dr" and USE_DR:
                            stride = e["offs"][1] - e["offs"][0]
                            rhs = AP(rt[:, 0:1].tensor, o1,
                                     [[RW, 128], [stride, 2], [1, ne]])
                            lhs = wsb[:, cur:cur + 128].rearrange(
                                "p (i m) -> p i m", m=64)
                            nc.tensor.matmul(
                                psl, lhs, rhs,
                                start=(ei == 0), stop=(ei == len(ENTRIES) - 1),
                                perf_mode=mybir.MatmulPerfMode.DoubleRow,
                                tile_position=pos)
                            cur += 128
                        elif e["kind"] == "dr":
                            for si in range(2):
                                oo = y0 * WP + e["offs"][si]
                                nc.tensor.matmul(
                                    psl, wsb[:, cur:cur + 64],
                                    rt[:, oo:oo + ne],
                                    start=(ei == 0 and si == 0),
                                    stop=(ei == len(ENTRIES) - 1 and si == 1),
                                    tile_position=pos)
                                cur += 64
                        else:
                            nc.tensor.matmul(
                                psl, wsb[:, cur:cur + 64], rt[:, o1:o1 + ne],
                                start=(ei == 0), stop=(ei == len(ENTRIES) - 1),
                                tile_position=pos)
                            cur += 64
                    outs.append((y0, nr, ne, ci))
                for (y0, nr, ne, ci) in outs:
                    oe = opool.tile([64, 462], F32, tag="oe")
                    src = ps[0:64, 0:ne] if ci == 0 else ps[64:128, 0:ne]
                    nc.scalar.copy(oe[:, 0:ne], src)
                    oev = oe[:, 0:ne].rearrange("p (r w) -> p r w", w=WP)
                    nc.sync.dma_start(out=out_ext[:, y0:y0 + nr, :],
                                      in_=oev[:, :, 1:65])

            emit_strip(0)
            emit_pair(0)
            emit_strip(1)
            emit_pair(1)
            emit_strip(2)
            emit_pair(2)
            emit_strip(3)
            emit_pair(3)
            emit_pair(4)
    nc.finalize()
    return nc


def kernel(x, base_weight, spline_weight, spline_scaler):
    x = np.ascontiguousarray(np.asarray(x, dtype=np.float32))
    wk = _fold_weights(np.asarray(base_weight), np.asarray(spline_weight),
                       np.asarray(spline_scaler))
    ct = _ctab()

    if "nc" not in _CACHE:
        _CACHE["nc"] = _build_nc()
    nc = _CACHE["nc"]

    in_maps = [{"x_img": x[i], "wk": wk, "ct": ct} for i in range(B)]
    res = run_bass_kernel_spmd(nc, in_maps, list(range(B)))
    _CACHE["last_res"] = res
    out = np.stack([res.results[i]["out"] for i in range(B)], axis=0)
    return out.astype(np.float32)


if __name__ == "__main__":
    rng = np.random.default_rng(0)
    ins = {
        "x": rng.standard_normal((B, C, H, W), dtype=np.float32),
        "base_weight": (rng.standard_normal((OUT_CH, 576)) * 0.05).astype(np.float32),
        "spline_weight": (rng.standard_normal((OUT_CH, 576, NJ)) * 0.05).astype(np.float32),
        "spline_scaler": (rng.standard_normal((OUT_CH, 576)) * 0.05).astype(np.float32),
    }
    o = kernel(**ins)
    print("kernel out:", o.shape, o.dtype, float(np.abs(o).max()))
